# revision 18
# baseline (speedup 1.0000x reference)
"""BlockDiffusionDecoder (mBART-style 2-layer decoder + BD3LM self-attn mask)
on 8 Trainium2 NeuronCores.

Sharding: sequence-parallel.  Core c owns batch b = c//2 and token half
h = c%2 (512 of the 1024 tokens).  Each core carries the FULL hidden state
in a uniform local layout: local tiles 0-3 = its own token half, local
tiles 4-7 = the global second half (x0 tokens).  For odd cores the two
regions coincide (own half duplicated), which keeps the SPMD program
identical across cores — only input data differs (ids/pos row order and
two 128x128 self-attention mask tiles).

Per layer each core computes K/V for all 8 local tiles but Q, attention,
cross-attention and FFN only for its own 4 tiles; sublayer deltas are
added straight into the fp32 residual (no collectives).  After each
non-final layer a single pair AllGather (1 MB bf16) refreshes local
tiles 4-7.  The LM head is token-parallel: full-vocab weights are
streamed from HBM and each core emits logits for its own 512 tokens
(bf16), so no final AllGather is needed.

Attention computes transposed scores (scores^T[key, query]) directly,
so softmax probabilities come out of the PE already in the layout AV
needs: no per-head transposes.  Additive masks are accumulated into the
score PSUM via identity-matmuls (finite -6e4 for "masked" so 0*mask
stays 0), exp runs straight from PSUM (scores are O(3), no max
subtraction needed), row sums come from ones-vector matmuls, and the
1/sum renormalization is folded into the PSUM->SBUF eviction.

LayerNorm scale/bias are folded into the downstream projection weights /
biases host-side, so on-device LN is just (x - mu) * rsqrt(var + eps).
"""
import sys

if "/opt/trn_rl_repo" not in sys.path:
    sys.path.insert(0, "/opt/trn_rl_repo")

import contextlib

import ml_dtypes
import numpy as np

import concourse.bass as bass
import concourse.bacc as bacc
import concourse.tile as tile
from concourse import mybir
from concourse.bass_utils import run_bass_kernel_spmd
from concourse.masks import make_identity

P = 128
B, D, H, NL, DFF, V, S = 4, 1024, 16, 2, 4096, 32000, 128
T = 1024
T2 = 512             # tokens owned per core
HD = D // H          # 64
BLK = 4
VP = 32768           # padded vocab
NT = 8               # local token tiles (4 own + 4 "x0 region")
NTO = 4              # own token tiles
ND = D // P          # 8 feature tiles
MH = ND              # 8 m-tiles for q/k (2 heads per tile)
NVC = VP // 512      # 64 lm-head 512-vocab chunks
NVG = VP // 1024     # 32 lm-head dma chunks
EMB_SCALE = 32.0     # sqrt(D)
MNEG = -60000.0      # finite "masked" additive value (exp -> 0)
BF = ml_dtypes.bfloat16

f32 = mybir.dt.float32
bf16 = mybir.dt.bfloat16
i32 = mybir.dt.int32
AF = mybir.ActivationFunctionType
ALU = mybir.AluOpType
AX = mybir.AxisListType

PAIRS = [[0, 1], [2, 3], [4, 5], [6, 7]]


def _rhs_tile(w_t: np.ndarray, nchunk: int) -> np.ndarray:
    """[d_in, d_out] -> [n_chunks, 128, k_tiles, nchunk] bf16."""
    d_in, d_out = w_t.shape
    kt = d_in // P
    nc_ = d_out // nchunk
    return np.ascontiguousarray(
        w_t.reshape(kt, P, nc_, nchunk).transpose(2, 1, 0, 3).astype(BF))


def _mask_consts():
    i = np.arange(P)
    diag = np.where((i[:, None] // BLK) == (i[None, :] // BLK), 0.0, MNEG)
    tri_s = np.where((i[:, None] // BLK) > (i[None, :] // BLK), 0.0, MNEG)
    tri_i = np.where((i[:, None] // BLK) >= (i[None, :] // BLK), 0.0, MNEG)
    return (diag.astype(np.float32), tri_s.astype(np.float32),
            tri_i.astype(np.float32))


def _assemble(out_core: np.ndarray) -> np.ndarray:
    """Kernel out [NVC*NTO*P, 512] -> [T2, VP] logits."""
    return (out_core.reshape(NVC, NTO * P, 512)
            .transpose(1, 0, 2).reshape(T2, VP))


def host_prepare(inputs: dict):
    ids = np.asarray(inputs["input_ids"])
    enc = np.asarray(inputs["enc_hidden"], dtype=np.float32)
    emask = np.asarray(inputs["enc_mask"])
    emb = np.ascontiguousarray(np.asarray(inputs["embed_tokens"], np.float32))
    pos = np.asarray(inputs["pos_embed"], np.float32)
    attn_w = np.asarray(inputs["attn_w"], np.float32)
    attn_b = np.asarray(inputs["attn_b"], np.float32)
    ln_w = np.asarray(inputs["ln_w"], np.float32)
    ln_b = np.asarray(inputs["ln_b"], np.float32)
    fc1_w = np.asarray(inputs["fc1_w"], np.float32)
    fc1_b = np.asarray(inputs["fc1_b"], np.float32)
    fc2_w = np.asarray(inputs["fc2_w"], np.float32)
    fc2_b = np.asarray(inputs["fc2_b"], np.float32)
    lm_w = np.asarray(inputs["lm_head_w"], np.float32)
    fs = np.asarray(inputs["final_ln_s"], np.float32)
    fb = np.asarray(inputs["final_ln_b"], np.float32)

    # final LN fold into lm head
    lm_pad = np.zeros((VP, D), np.float32)
    lm_pad[: V + 1] = lm_w
    lm_t = lm_pad.T * fs[:, None]                      # [D, VP]
    logit_bias = fb @ lm_pad.T                         # [VP] host-added
    wlm = np.ascontiguousarray(
        lm_t.reshape(ND, P, NVG, 1024).transpose(2, 1, 0, 3).astype(BF))

    mdiag, mtris, mtrii = _mask_consts()
    allneg = np.full((P, P), MNEG, np.float32)

    # shared (core-independent) weight tensors
    shared = {"emb": emb, "wlm": wlm,
              "lnemb": np.stack([np.asarray(inputs["ln_emb_s"], np.float32),
                                 np.asarray(inputs["ln_emb_b"], np.float32)])}
    for l in range(NL):
        for a, tag in ((0, "s"), (1, "c")):
            wq, wk, wv, wo = attn_w[l, a]
            bq, bk, bv, bo = attn_b[l, a]
            s_ln = ln_w[l, a]
            b_ln = ln_b[l, a]
            # fold LN affine into x-side projections (q always; k,v only
            # for self-attn where they read the LN'd x)
            wqT = wq.T * s_ln[:, None]
            bq_e = bq + b_ln @ wq.T
            if a == 0:
                wkT = wk.T * s_ln[:, None]
                bk_e = bk + b_ln @ wk.T
                wvT = wv.T * s_ln[:, None]
                bv_e = bv + b_ln @ wv.T
            else:
                wkT, bk_e, wvT, bv_e = wk.T, bk, wv.T, bv
            shared[f"wq{tag}{l}"] = _rhs_tile(wqT, P)
            shared[f"wk{tag}{l}"] = _rhs_tile(wkT, P if a == 0 else 512)
            shared[f"wv{tag}{l}"] = _rhs_tile(wvT, 512)
            shared[f"wo{tag}{l}"] = _rhs_tile(wo.T, 512)
            shared[f"bq{tag}{l}"] = np.ascontiguousarray(
                bq_e.reshape(MH, P).T)
            shared[f"bk{tag}{l}"] = np.ascontiguousarray(
                bk_e.reshape(MH, P).T)
            shared[f"bv{tag}{l}"] = bv_e.reshape(1, D).copy()
            shared[f"bo{tag}{l}"] = bo.reshape(1, D).copy()
        s3, b3 = ln_w[l, 2], ln_b[l, 2]
        f1T = fc1_w[l].T * s3[:, None]
        bf1_e = fc1_b[l] + b3 @ fc1_w[l].T
        shared[f"wf1{l}"] = _rhs_tile(f1T, P)
        shared[f"bf1{l}"] = np.ascontiguousarray(
            bf1_e.reshape(DFF // P, P).T)
        # fc2 streamed as [kk-group of 8][n2] chunks: [4, 2, P, 8, 512]
        shared[f"wf2{l}"] = np.ascontiguousarray(
            fc2_w[l].T.reshape(4, 8, P, 2, 512)
            .transpose(0, 3, 2, 1, 4).astype(BF))
        shared[f"bf2{l}"] = fc2_b[l].reshape(1, D).copy()

    maps = []
    for c in range(8):
        b_, half = c // 2, c % 2
        own = slice(half * T2, half * T2 + T2)
        x0 = slice(T2, T)
        m = dict(shared)
        m["ids"] = np.concatenate(
            [ids[b_, own], ids[b_, x0]]).reshape(T, 1).astype(np.int32)
        m["pos"] = np.ascontiguousarray(
            np.concatenate([pos[own], pos[x0]], axis=0))
        m["encT"] = np.ascontiguousarray(enc[b_].T.astype(BF))
        m["cmaskT"] = np.ascontiguousarray(
            ((1.0 - emask[b_].astype(np.float32)) * MNEG).reshape(S, 1))
        mA = mdiag if half == 0 else mtrii
        mB = mtris if half == 0 else allneg
        m["mAT"] = np.ascontiguousarray(mA.T.astype(BF))
        m["mBT"] = np.ascontiguousarray(mB.T.astype(BF))
        maps.append(m)
    return maps, logit_bias


def build_nc(collectives=True, gelu=AF.Gelu_apprx_tanh):
    nc = bacc.Bacc(num_devices=8 if collectives else None, trn_type="TRN2")

    ids_d = nc.dram_tensor("ids", [T, 1], i32, kind="ExternalInput")
    emb_d = nc.dram_tensor("emb", [V + 1, D], f32, kind="ExternalInput")
    pos_d = nc.dram_tensor("pos", [T, D], f32, kind="ExternalInput")
    encT_d = nc.dram_tensor("encT", [D, S], bf16, kind="ExternalInput")
    cmaskT_d = nc.dram_tensor("cmaskT", [S, 1], f32, kind="ExternalInput")
    lnemb_d = nc.dram_tensor("lnemb", [2, D], f32, kind="ExternalInput")
    mAT_d = nc.dram_tensor("mAT", [P, P], bf16, kind="ExternalInput")
    mBT_d = nc.dram_tensor("mBT", [P, P], bf16, kind="ExternalInput")
    wlm_d = nc.dram_tensor("wlm", [NVG, P, ND, 1024], bf16,
                           kind="ExternalInput")
    wd, bd = {}, {}
    for l in range(NL):
        for tg in ("s", "c"):
            wd[f"wq{tg}{l}"] = nc.dram_tensor(
                f"wq{tg}{l}", [MH, P, ND, P], bf16, kind="ExternalInput")
            wk_shape = ([MH, P, ND, P] if tg == "s"
                        else [2, P, ND, 512])
            wd[f"wk{tg}{l}"] = nc.dram_tensor(
                f"wk{tg}{l}", wk_shape, bf16, kind="ExternalInput")
            wd[f"wv{tg}{l}"] = nc.dram_tensor(
                f"wv{tg}{l}", [2, P, ND, 512], bf16, kind="ExternalInput")
            wd[f"wo{tg}{l}"] = nc.dram_tensor(
                f"wo{tg}{l}", [2, P, ND, 512], bf16, kind="ExternalInput")
            bd[f"bq{tg}{l}"] = nc.dram_tensor(
                f"bq{tg}{l}", [P, MH], f32, kind="ExternalInput")
            bd[f"bk{tg}{l}"] = nc.dram_tensor(
                f"bk{tg}{l}", [P, MH], f32, kind="ExternalInput")
            bd[f"bv{tg}{l}"] = nc.dram_tensor(
                f"bv{tg}{l}", [1, D], f32, kind="ExternalInput")
            bd[f"bo{tg}{l}"] = nc.dram_tensor(
                f"bo{tg}{l}", [1, D], f32, kind="ExternalInput")
        wd[f"wf1{l}"] = nc.dram_tensor(
            f"wf1{l}", [DFF // P, P, ND, P], bf16, kind="ExternalInput")
        bd[f"bf1{l}"] = nc.dram_tensor(
            f"bf1{l}", [P, DFF // P], f32, kind="ExternalInput")
        wd[f"wf2{l}"] = nc.dram_tensor(
            f"wf2{l}", [4, 2, P, 8, 512], bf16, kind="ExternalInput")
        bd[f"bf2{l}"] = nc.dram_tensor(
            f"bf2{l}", [1, D], f32, kind="ExternalInput")
    out_d = nc.dram_tensor("out", [NVC * NTO * P, 512], bf16,
                           kind="ExternalOutput")

    def bcast(ap_1d, p=P):
        return bass.AP(tensor=ap_1d.tensor, offset=ap_1d.offset,
                       ap=[[0, p]] + list(ap_1d.ap))

    with tile.TileContext(nc) as tc:
        gctx = contextlib.ExitStack()
        with gctx:
            consts = gctx.enter_context(tc.tile_pool(name="consts", bufs=1))
            small = gctx.enter_context(tc.tile_pool(name="small", bufs=4))
            sp = gctx.enter_context(tc.tile_pool(name="sp", bufs=2))
            xb = gctx.enter_context(tc.tile_pool(name="xb", bufs=2))
            dram = gctx.enter_context(
                tc.tile_pool(name="dram", bufs=1, space="DRAM"))
            ps_a = gctx.enter_context(
                tc.tile_pool(name="ps_a", bufs=4, space="PSUM"))
            ps_av = gctx.enter_context(
                tc.tile_pool(name="ps_av", bufs=3, space="PSUM"))
            ps_tr = gctx.enter_context(
                tc.tile_pool(name="ps_tr", bufs=1, space="PSUM"))
            glob = gctx.enter_context(tc.tile_pool(name="glob", bufs=1))

            ident = consts.tile([P, P], bf16)
            make_identity(nc, ident[:])
            eps_t = consts.tile([P, 1], f32)
            nc.vector.memset(eps_t[:], 1e-5)
            ones_t = consts.tile([P, 64], bf16)
            nc.vector.memset(ones_t[:], 1.0)
            mAT = consts.tile([P, P], bf16)
            nc.sync.dma_start(out=mAT[:], in_=mAT_d[:])
            mBT = consts.tile([P, P], bf16)
            nc.sync.dma_start(out=mBT[:], in_=mBT_d[:])
            cmaskT = consts.tile([S, 1], f32)
            nc.sync.dma_start(out=cmaskT[:], in_=cmaskT_d[:])

            # final hidden (transposed) for the LM head — outlives the
            # stack pools
            hT = glob.tile([P, ND, T2], bf16)

            ccx_in = dram.tile([T2, D], bf16, name="ccx_in")
            ccx_out = (dram.tile([2, T2, D], bf16, name="ccx_out")
                       if collectives else None)

            def ln_stats(src_ap):
                """-> (negmur [P,1], rstd [P,1]): x_hat = x*rstd + negmur."""
                st = small.tile([P, 2, 6], f32, name="lnstats")
                nc.vector.bn_stats(out=st[:, 0, :], in_=src_ap[:, 0:512])
                nc.vector.bn_stats(out=st[:, 1, :], in_=src_ap[:, 512:1024])
                mv = small.tile([P, 2], f32, name="lnmv")
                nc.vector.bn_aggr(out=mv[:], in_=st[:])
                rstd = small.tile([P, 1], f32, name="lnrstd")
                nc.scalar.activation(out=rstd[:], in_=mv[:, 1:2],
                                     func=AF.Sqrt, bias=eps_t[:])
                nc.vector.reciprocal(out=rstd[:], in_=rstd[:])
                negmur = small.tile([P, 1], f32, name="lnnm")
                nc.vector.tensor_scalar(out=negmur[:], in0=mv[:, 0:1],
                                        scalar1=rstd[:], scalar2=-1.0,
                                        op0=ALU.mult, op1=ALU.mult)
                return negmur, rstd

            def ln_pure(src_ap, dst_ap, par):
                """dst = (src - mean) * rsqrt(var+eps); engine by parity."""
                negmur, rstd = ln_stats(src_ap)
                if par % 2 == 0:
                    nc.scalar.activation(out=dst_ap, in_=src_ap,
                                         func=AF.Identity,
                                         scale=rstd[:], bias=negmur[:])
                else:
                    nc.vector.tensor_scalar(out=dst_ap, in0=src_ap,
                                            scalar1=rstd[:],
                                            scalar2=negmur[:],
                                            op0=ALU.mult, op1=ALU.add)

            # ---------------- stack phase ----------------
            sctx = contextlib.ExitStack()
            with sctx:
                pers = sctx.enter_context(tc.tile_pool(name="pers", bufs=1))
                wp = sctx.enter_context(tc.tile_pool(name="wp", bufs=3))
                lnp = sctx.enter_context(tc.tile_pool(name="lnp", bufs=1))

                h = pers.tile([P, NT, D], f32)
                encT = pers.tile([P, ND, S], bf16)
                nc.sync.dma_start(
                    out=encT[:],
                    in_=encT_d.rearrange("(k p) s -> p k s", p=P))
                # cross-attn K/V for both layers, precomputed
                encKT = pers.tile([P, NL, MH, S], bf16)
                encV = pers.tile([P, NL, D], bf16)

                def bias_bcast(src_row, name):
                    t = lnp.tile([P, D], bf16, name=name, tag="bb", bufs=2)
                    nc.gpsimd.dma_start(out=t[:], in_=bcast(src_row))
                    return t

                def transpose_to(dst_ap, src_ap, par):
                    tp_ = ps_tr.tile([P, P], bf16, name="trps")
                    nc.tensor.transpose(tp_[:], src_ap, ident[:])
                    if par % 2 == 0:
                        nc.scalar.activation(out=dst_ap, in_=tp_[:],
                                             func=AF.Copy)
                    else:
                        nc.vector.tensor_copy(out=dst_ap, in_=tp_[:])

                # ---- embed + emb LN (general affine) ----
                with nc.named_scope("embed"):
                    lnes = bias_bcast(lnemb_d[0], "lnes")
                    lneb = bias_bcast(lnemb_d[1], "lneb")
                    for tt in range(NT):
                        idt = small.tile([P, 1], i32, name="idt")
                        nc.sync.dma_start(out=idt[:],
                                          in_=ids_d[tt * P:(tt + 1) * P])
                        g = xb.tile([P, D], f32, name="xrow", tag="xf",
                                    bufs=4)
                        nc.gpsimd.indirect_dma_start(
                            out=g[:], out_offset=None, in_=emb_d[:],
                            in_offset=bass.IndirectOffsetOnAxis(
                                ap=idt[:, :1], axis=0))
                        pt = xb.tile([P, D], f32, name="xrow2", tag="xf",
                                     bufs=4)
                        nc.sync.dma_start(out=pt[:],
                                          in_=pos_d[tt * P:(tt + 1) * P])
                        nc.vector.tensor_scalar(out=g[:], in0=g[:],
                                                scalar1=EMB_SCALE,
                                                scalar2=None, op0=ALU.mult)
                        nc.vector.tensor_tensor(out=g[:], in0=g[:],
                                                in1=pt[:], op=ALU.add)
                        negmur, rstd = ln_stats(g[:])
                        nc.scalar.activation(out=pt[:], in_=g[:],
                                             func=AF.Identity,
                                             scale=rstd[:],
                                             bias=negmur[:])
                        nc.vector.tensor_tensor(out=pt[:], in0=pt[:],
                                                in1=lnes[:], op=ALU.mult)
                        nc.vector.tensor_tensor(out=h[:, tt, :], in0=pt[:],
                                                in1=lneb[:], op=ALU.add)

                    # enc K/V for both layers (independent of h)
                    for l in range(NL):
                        bk2 = small.tile([P, MH], f32, name="bk2")
                        nc.sync.dma_start(out=bk2[:], in_=bd[f"bkc{l}"][:])
                        bv2 = bias_bcast(bd[f"bvc{l}"][0], "bv2")
                        kraw = sp.tile([S, D], bf16, name="kraw", bufs=1)
                        for hf in range(2):
                            wck = wp.tile([P, ND, 512], bf16, name="wch",
                                          tag="wch")
                            nc.sync.dma_start(out=wck[:],
                                              in_=wd[f"wkc{l}"][hf])
                            psk = ps_a.tile([P, 512], f32, name="psq")
                            for k in range(ND):
                                nc.tensor.matmul(
                                    out=psk[:], lhsT=encT[:, k, :],
                                    rhs=wck[:, k, :],
                                    start=(k == 0), stop=(k == ND - 1))
                            nc.scalar.activation(
                                out=kraw[:, hf * 512:(hf + 1) * 512],
                                in_=psk[:], func=AF.Copy)
                        for mq in range(MH):
                            tpk = ps_tr.tile([P, P], bf16, name="trps")
                            nc.tensor.transpose(
                                tpk[:], kraw[:, mq * P:(mq + 1) * P],
                                ident[:])
                            nc.vector.tensor_scalar(
                                out=encKT[:, l, mq, :], in0=tpk[:],
                                scalar1=bk2[:, mq:mq + 1], scalar2=None,
                                op0=ALU.add)
                        for hf in range(2):
                            wcv = wp.tile([P, ND, 512], bf16, name="wch",
                                          tag="wch")
                            nc.sync.dma_start(out=wcv[:],
                                              in_=wd[f"wvc{l}"][hf])
                            psv = ps_a.tile([P, 512], f32, name="psq")
                            for k in range(ND):
                                nc.tensor.matmul(
                                    out=psv[:], lhsT=encT[:, k, :],
                                    rhs=wcv[:, k, :],
                                    start=(k == 0), stop=(k == ND - 1))
                            nc.vector.tensor_tensor(
                                out=encV[:, l, hf * 512:(hf + 1) * 512],
                                in0=psv[:],
                                in1=bv2[:, hf * 512:(hf + 1) * 512],
                                op=ALU.add)

                def proj_qk(dst, w_key, b_sb, scale, src_xT, ncols):
                    """dst[:, m, 0:ncols] = (x @ W)^T + b, x = src_xT cols."""
                    for m in range(MH):
                        wch = wp.tile([P, ND, P], bf16, name="wch", tag="wch")
                        nc.sync.dma_start(out=wch[:], in_=wd[w_key][m])
                        for cb in range(ncols // 512):
                            psq = ps_a.tile([P, 512], f32, name="psq")
                            for k in range(ND):
                                nc.tensor.matmul(
                                    out=psq[:], lhsT=wch[:, k, :],
                                    rhs=src_xT[:, k,
                                               cb * 512:(cb + 1) * 512],
                                    start=(k == 0), stop=(k == ND - 1))
                            if scale is None:
                                nc.vector.tensor_scalar(
                                    out=dst[:, m, cb * 512:(cb + 1) * 512],
                                    in0=psq[:], scalar1=b_sb[:, m:m + 1],
                                    scalar2=None, op0=ALU.add)
                            else:
                                nc.vector.tensor_scalar(
                                    out=dst[:, m, cb * 512:(cb + 1) * 512],
                                    in0=psq[:], scalar1=b_sb[:, m:m + 1],
                                    scalar2=scale, op0=ALU.add, op1=ALU.mult)

                def oproj_update(src_oT, wo_key, bo_key):
                    """h[own] += oT @ Wo + bo (bias via pre-add)."""
                    bo_b = bias_bcast(bd[bo_key][0], "bo_b")
                    for tt in range(NTO):
                        nc.vector.tensor_tensor(out=h[:, tt, :],
                                                in0=h[:, tt, :],
                                                in1=bo_b[:], op=ALU.add)
                    wchs = []
                    for n2 in range(2):
                        wch = wp.tile([P, ND, 512], bf16, name="wch",
                                      tag="wch")
                        nc.sync.dma_start(out=wch[:], in_=wd[wo_key][n2])
                        wchs.append(wch)
                    for tt in range(NTO):
                        for n2 in range(2):
                            pso = ps_a.tile([P, 512], f32, name="psq")
                            for k in range(ND):
                                nc.tensor.matmul(
                                    out=pso[:],
                                    lhsT=src_oT[:, k, tt * P:(tt + 1) * P],
                                    rhs=wchs[n2][:, k, :],
                                    start=(k == 0), stop=(k == ND - 1))
                            nc.vector.tensor_tensor(
                                out=h[:, tt, n2 * 512:(n2 + 1) * 512],
                                in0=h[:, tt, n2 * 512:(n2 + 1) * 512],
                                in1=pso[:], op=ALU.add)

                for l in range(NL):
                  with nc.named_scope(f"layer{l}"):
                    # ======== self attention ========
                    xT = pers.tile([P, ND, T], bf16, name="xT", tag="xT")
                    for tt in range(NT):
                        xt_ = xb.tile([P, D], bf16, name="xbrow", tag="xh")
                        ln_pure(h[:, tt, :], xt_[:], tt)
                        for k in range(ND):
                            transpose_to(xT[:, k, tt * P:(tt + 1) * P],
                                         xt_[:, k * P:(k + 1) * P], k)

                    qT = pers.tile([P, MH, T2], bf16, name="qT", tag="qT")
                    kT = pers.tile([P, MH, T], bf16, name="kT", tag="kT")
                    vv = pers.tile([P, NT, D], bf16, name="vv", tag="vv")
                    bq_sb = small.tile([P, MH], f32, name="bq")
                    nc.sync.dma_start(out=bq_sb[:], in_=bd[f"bqs{l}"][:])
                    bk_sb = small.tile([P, MH], f32, name="bk")
                    nc.sync.dma_start(out=bk_sb[:], in_=bd[f"bks{l}"][:])
                    bv_b = bias_bcast(bd[f"bvs{l}"][0], "bv_b")

                    proj_qk(qT, f"wqs{l}", bq_sb, 0.125, xT, T2)
                    proj_qk(kT, f"wks{l}", bk_sb, None, xT, T)
                    for hf in range(2):
                        wch_v = wp.tile([P, ND, 512], bf16, name="wch",
                                        tag="wch")
                        nc.sync.dma_start(out=wch_v[:], in_=wd[f"wvs{l}"][hf])
                        for tt in range(NT):
                            psv = ps_a.tile([P, 512], f32, name="psq")
                            for k in range(ND):
                                nc.tensor.matmul(
                                    out=psv[:],
                                    lhsT=xT[:, k, tt * P:(tt + 1) * P],
                                    rhs=wch_v[:, k, :],
                                    start=(k == 0), stop=(k == ND - 1))
                            nc.vector.tensor_tensor(
                                out=vv[:, tt, hf * 512:(hf + 1) * 512],
                                in0=psv[:],
                                in1=bv_b[:, hf * 512:(hf + 1) * 512],
                                op=ALU.add)

                    # transposed scores + softmax + AV per head
                    oT = pers.tile([P, ND, T2], bf16, name="oT", tag="oT")
                    for hl in range(H):
                        prow = slice((hl % 2) * 64, (hl % 2) * 64 + 64)
                        mq = hl // 2
                        hds = slice(hl * HD, (hl + 1) * HD)
                        ptile = sp.tile([P, 5, T2], bf16, name="ptile")
                        # diag tiles (slot 4): scoresT quarters + mask
                        psD = ps_a.tile([P, 512], f32, name="psq")
                        for qi in range(NTO):
                            cs = slice(qi * P, (qi + 1) * P)
                            nc.tensor.matmul(
                                out=psD[:, cs],
                                lhsT=kT[prow, mq, qi * P:(qi + 1) * P],
                                rhs=qT[prow, mq, qi * P:(qi + 1) * P],
                                start=(qi == 0), stop=False)
                            nc.tensor.matmul(
                                out=psD[:, cs], lhsT=ident[:], rhs=mAT[:],
                                start=False, stop=(qi == NTO - 1))
                        nc.scalar.activation(out=ptile[:, 4, :], in_=psD[:],
                                             func=AF.Exp)
                        # strip tiles (slots 0-3): key tile 4+j covers
                        # queries j..3; the leading block of each strip
                        # (query tile j) is masked with mBT.  Strips 0,1
                        # get their own bank; strips 2 (256 wide) and 3
                        # (128 wide) pack into one bank at offsets 0/256.
                        for j in range(2):
                            w_ = (NTO - j) * P
                            psS = ps_a.tile([P, 512], f32, name="psq")
                            nc.tensor.matmul(
                                out=psS[:, 0:w_],
                                lhsT=kT[prow, mq, (4 + j) * P:(5 + j) * P],
                                rhs=qT[prow, mq, j * P:T2],
                                start=True, stop=False)
                            nc.tensor.matmul(
                                out=psS[:, 0:P], lhsT=ident[:],
                                rhs=mBT[:], start=False, stop=True)
                            nc.scalar.activation(
                                out=ptile[:, j, j * P:T2],
                                in_=psS[:, 0:w_], func=AF.Exp)
                        psS = ps_a.tile([P, 512], f32, name="psq")
                        for j, off in ((2, 0), (3, 256)):
                            w_ = (NTO - j) * P
                            nc.tensor.matmul(
                                out=psS[:, off:off + w_],
                                lhsT=kT[prow, mq, (4 + j) * P:(5 + j) * P],
                                rhs=qT[prow, mq, j * P:T2],
                                start=(j == 2), stop=False)
                            nc.tensor.matmul(
                                out=psS[:, off:off + P], lhsT=ident[:],
                                rhs=mBT[:], start=False, stop=(j == 3))
                        for j, off in ((2, 0), (3, 256)):
                            w_ = (NTO - j) * P
                            nc.scalar.activation(
                                out=ptile[:, j, j * P:T2],
                                in_=psS[:, off:off + w_], func=AF.Exp)

                        sums_ps = ps_a.tile([P, 512], f32, name="psq")
                        nc.tensor.matmul(out=sums_ps[0:1, :],
                                         lhsT=ones_t[:, 0:1],
                                         rhs=ptile[:, 4, :],
                                         start=True, stop=False)
                        for j in range(NTO):
                            nc.tensor.matmul(
                                out=sums_ps[0:1, j * P:T2],
                                lhsT=ones_t[:, 0:1],
                                rhs=ptile[:, j, j * P:T2],
                                start=False, stop=(j == NTO - 1))
                        recip_sb = small.tile([1, 512], bf16, name="recip",
                                              bufs=4)
                        with nc.allow_low_precision(
                                reason="softmax 1/sum in bf16"):
                            nc.vector.reciprocal(out=recip_sb[:],
                                                 in_=sums_ps[0:1, :])
                        rb_ps = ps_av.tile([64, 512], f32, name="pav")
                        nc.tensor.matmul(out=rb_ps[:], lhsT=ones_t[0:1, :],
                                         rhs=recip_sb[:], start=True,
                                         stop=True)
                        recip_b = xb.tile([64, 512], bf16, name="rbb",
                                          tag="rb", bufs=4)
                        nc.scalar.activation(out=recip_b[:], in_=rb_ps[:],
                                             func=AF.Copy)
                        pav = ps_av.tile([64, T2], f32, name="pav")
                        for qi in range(NTO):
                            nc.tensor.matmul(
                                out=pav[:, qi * P:(qi + 1) * P],
                                lhsT=vv[:, qi, hds],
                                rhs=ptile[:, 4, qi * P:(qi + 1) * P],
                                start=(qi == 0), stop=False)
                        for j in range(NTO):
                            nc.tensor.matmul(
                                out=pav[:, j * P:T2],
                                lhsT=vv[:, 4 + j, hds],
                                rhs=ptile[:, j, j * P:T2],
                                start=False, stop=(j == NTO - 1))
                        nc.vector.tensor_tensor(out=oT[prow, mq, :],
                                                in0=pav[:], in1=recip_b[:],
                                                op=ALU.mult)
                    oproj_update(oT, f"wos{l}", f"bos{l}")

                    # ======== cross attention ========
                    x2T = pers.tile([P, ND, T2], bf16, name="x2T", tag="x2T")
                    for ti in range(NTO):
                        xt_ = xb.tile([P, D], bf16, name="xbrow", tag="xh")
                        ln_pure(h[:, ti, :], xt_[:], ti)
                        for k in range(ND):
                            transpose_to(x2T[:, k, ti * P:(ti + 1) * P],
                                         xt_[:, k * P:(k + 1) * P], k)
                    q2T = pers.tile([P, MH, T2], bf16, name="q2T", tag="qT")
                    bq2_sb = small.tile([P, MH], f32, name="bq2")
                    nc.sync.dma_start(out=bq2_sb[:], in_=bd[f"bqc{l}"][:])
                    proj_qk(q2T, f"wqc{l}", bq2_sb, 0.125, x2T, T2)

                    o2T = pers.tile([P, ND, T2], bf16, name="o2T", tag="oT")
                    for hl in range(H):
                        prow = slice((hl % 2) * 64, (hl % 2) * 64 + 64)
                        mq = hl // 2
                        hds = slice(hl * HD, (hl + 1) * HD)
                        p2tile = sp.tile([S, T2], bf16, name="p2tile",
                                         bufs=3)
                        ps2 = ps_a.tile([P, 512], f32, name="psq")
                        nc.tensor.matmul(out=ps2[:],
                                         lhsT=encKT[prow, l, mq, :],
                                         rhs=q2T[prow, mq, :],
                                         start=True, stop=True)
                        nc.scalar.activation(out=p2tile[:], in_=ps2[:],
                                             func=AF.Exp, bias=cmaskT[:])
                        sums_ps = ps_a.tile([P, 512], f32, name="psq")
                        nc.tensor.matmul(out=sums_ps[0:1, :],
                                         lhsT=ones_t[:, 0:1],
                                         rhs=p2tile[:], start=True,
                                         stop=True)
                        recip_sb = small.tile([1, 512], bf16, name="recip",
                                              bufs=4)
                        with nc.allow_low_precision(
                                reason="softmax 1/sum in bf16"):
                            nc.vector.reciprocal(out=recip_sb[:],
                                                 in_=sums_ps[0:1, :])
                        rb_ps = ps_av.tile([64, 512], f32, name="pav")
                        nc.tensor.matmul(out=rb_ps[:], lhsT=ones_t[0:1, :],
                                         rhs=recip_sb[:], start=True,
                                         stop=True)
                        recip_b = xb.tile([64, 512], bf16, name="rbb",
                                          tag="rb", bufs=4)
                        nc.scalar.activation(out=recip_b[:], in_=rb_ps[:],
                                             func=AF.Copy)
                        pav2 = ps_av.tile([64, T2], f32, name="pav")
                        nc.tensor.matmul(out=pav2[:], lhsT=encV[:, l, hds],
                                         rhs=p2tile[:], start=True,
                                         stop=True)
                        nc.vector.tensor_tensor(out=o2T[prow, mq, :],
                                                in0=pav2[:], in1=recip_b[:],
                                                op=ALU.mult)
                    oproj_update(o2T, f"woc{l}", f"boc{l}")

                    # ======== FFN ========
                    x3T = pers.tile([P, ND, T2], bf16, name="x3T", tag="x2T")
                    for ti in range(NTO):
                        xt_ = xb.tile([P, D], bf16, name="xbrow", tag="xh")
                        ln_pure(h[:, ti, :], xt_[:], ti)
                        for k in range(ND):
                            transpose_to(x3T[:, k, ti * P:(ti + 1) * P],
                                         xt_[:, k * P:(k + 1) * P], k)
                    bf1_sb = small.tile([P, DFF // P], f32, name="bf1s")
                    nc.sync.dma_start(out=bf1_sb[:], in_=bd[f"bf1{l}"][:])
                    bf2_b = bias_bcast(bd[f"bf2{l}"][0], "bf2_b")
                    for tt in range(NTO):
                        nc.vector.tensor_tensor(out=h[:, tt, :],
                                                in0=h[:, tt, :],
                                                in1=bf2_b[:], op=ALU.add)
                    for dh in range(2):
                        gT = pers.tile([P, 16, T2], bf16, name="gT",
                                       tag="xT")
                        for df in range(16):
                            dff = dh * 16 + df
                            f1c = wp.tile([P, ND, P], bf16, name="wch",
                                          tag="wch")
                            nc.sync.dma_start(out=f1c[:],
                                              in_=wd[f"wf1{l}"][dff])
                            psf = ps_a.tile([P, 512], f32, name="psq")
                            for k in range(ND):
                                nc.tensor.matmul(
                                    out=psf[:], lhsT=f1c[:, k, :],
                                    rhs=x3T[:, k, :],
                                    start=(k == 0), stop=(k == ND - 1))
                            nc.scalar.activation(
                                out=gT[:, df, :], in_=psf[:], func=gelu,
                                bias=bf1_sb[:, dff:dff + 1])
                        for n2 in range(2):
                            psf2s = [ps_a.tile([P, 512], f32, name="psq")
                                     for _ in range(NTO)]
                            for kg in range(2):
                                f2c = wp.tile([P, 8, 512], bf16, name="wch",
                                              tag="wch")
                                nc.sync.dma_start(
                                    out=f2c[:],
                                    in_=wd[f"wf2{l}"][dh * 2 + kg, n2])
                                for tt in range(NTO):
                                    for kk in range(8):
                                        nc.tensor.matmul(
                                            out=psf2s[tt][:],
                                            lhsT=gT[:, kg * 8 + kk,
                                                    tt * P:(tt + 1) * P],
                                            rhs=f2c[:, kk, :],
                                            start=(kg == 0 and kk == 0),
                                            stop=(kg == 1 and kk == 7))
                            for tt in range(NTO):
                                nc.vector.tensor_tensor(
                                    out=h[:, tt, n2 * 512:(n2 + 1) * 512],
                                    in0=h[:, tt, n2 * 512:(n2 + 1) * 512],
                                    in1=psf2s[tt][:], op=ALU.add)

                    # ======== pair exchange (not after last layer) ========
                    if l < NL - 1 and collectives:
                        for tl in range(NTO):
                            nc.gpsimd.dma_start(
                                out=ccx_in[tl * P:(tl + 1) * P, :],
                                in_=h[:, tl, :])
                        nc.gpsimd.collective_compute(
                            "AllGather", ALU.bypass, replica_groups=PAIRS,
                            ins=[ccx_in[:]], outs=[ccx_out[:]])
                        for tl in range(NTO):
                            nc.gpsimd.dma_start(
                                out=h[:, 4 + tl, :],
                                in_=ccx_out[1, tl * P:(tl + 1) * P, :])

                # ---- final LN (pure; affine folded into lm head) ----
                with nc.named_scope("final_ln"):
                    for tt in range(NTO):
                        xt_ = xb.tile([P, D], bf16, name="xbrow", tag="xh")
                        ln_pure(h[:, tt, :], xt_[:], tt)
                        for k in range(ND):
                            transpose_to(hT[:, k, tt * P:(tt + 1) * P],
                                         xt_[:, k * P:(k + 1) * P], k)
            # stack pools closed here

            # ---------------- LM head ----------------
            with nc.named_scope("lmhead"):
                lctx = contextlib.ExitStack()
                with lctx:
                    lmw = lctx.enter_context(tc.tile_pool(name="lmw",
                                                          bufs=3))
                    for vg in range(NVG):
                        wch = lmw.tile([P, ND, 1024], bf16, name="wlmc")
                        nc.sync.dma_start(out=wch[:], in_=wlm_d[vg])
                        for tt in range(NTO):
                            for hf in range(2):
                                vc = vg * 2 + hf
                                psl = ps_a.tile([P, 512], f32, name="psq")
                                for k in range(ND):
                                    nc.tensor.matmul(
                                        out=psl[:],
                                        lhsT=hT[:, k, tt * P:(tt + 1) * P],
                                        rhs=wch[:, k,
                                                hf * 512:(hf + 1) * 512],
                                        start=(k == 0), stop=(k == ND - 1))
                                osb = xb.tile([P, 512], bf16, name="ev512",
                                              tag="ev", bufs=6)
                                if (vc + tt) % 2 == 0:
                                    nc.scalar.activation(out=osb[:],
                                                         in_=psl[:],
                                                         func=AF.Copy)
                                else:
                                    nc.vector.tensor_copy(out=osb[:],
                                                          in_=psl[:])
                                ro = (vc * NTO + tt) * P
                                nc.sync.dma_start(
                                    out=out_d[ro:ro + P, :], in_=osb[:])
    nc.compile()
    return nc


_NC_CACHE = {}


def _get_nc(key=(True,)):
    if key not in _NC_CACHE:
        _NC_CACHE[key] = build_nc(collectives=key[0])
    return _NC_CACHE[key]


def kernel(**inputs) -> np.ndarray:
    nc = _get_nc()
    maps, logit_bias = host_prepare(inputs)
    res = run_bass_kernel_spmd(nc, maps, core_ids=list(range(8)),
                               trace=False)
    logits = np.empty((B, T, V + 1), np.float32)
    lb = logit_bias[: V + 1][None, :]
    for c in range(8):
        b_, half = c // 2, c % 2
        full = _assemble(res.results[c]["out"])
        logits[b_, half * T2:(half + 1) * T2] = (
            full[:, : V + 1].astype(np.float32) + lb)
    return logits
